# revision 29
# baseline (speedup 1.0000x reference)
"""BiLIF (bidirectional leaky-integrate-and-fire) node on 8 Trainium2 NeuronCores.

Problem: inputs [T=16, B=64, N=65536] f32.
  s1 = LIF-scan(x,          tau=4/3, v_th=0.75)   (hard reset to 0)
  s2 = LIF-scan(flip(x, 0), tau=4/3, v_th=1.25)
  out = (s1 + s2) / 2

Strategy
  - Shard the batch dim across the 8 cores (pure data parallel). Per core:
    8*65536 positions = 128 partitions x 4096 columns, two [128, 2048]
    column chunks. Both direction scans run concurrently: at step t the
    forward scan consumes x[t], the backward scan consumes x[15-t], so
    out[t] completes at step t and every x tile is loaded exactly once.
  - DVE does ONLY the two fused LIF step passes (charge+reset as one
    2-src custom op per direction per step -- the irreducible chain).
    The t=0 charges run on ACT (Copy with scale/bias) to keep DVE lean.
  - Direction 2 keeps a SHIFTED state g = h2 - 0.5 (the shift is folded
    into the custom op's three constants), so both directions spike at
    the SAME threshold 0.75. h1 and g share one [128, 4096] tile and ONE
    ACT Sign instruction produces both sigma tiles in fp8e4m3 (exact on
    {-1,0,1}).
  - PE combines AND packs 4-to-1: 8 accumulating 512-col matmuls per
    chunk-step (4 strips x 2 dirs) into one [128, 512] PSUM bank.
    Strip-s weights map partition 4q+j -> psum row 32s+q with weight
    4^j (powers of two: exact in fp8e4m3; base-3's 27 is NOT), over
    data cols [512s, 512s+512): psum = sum_j 4^j*(sig1+sig2)[4q+j],
    |psum| <= 170 integer. 4 data cols pack into one int8 byte ->
    output DMA is 0.25 B/elem (2.1 MB/core).
  - Four consecutive steps write the four banks of one psum tile; ONE
    ACT drain per 4 steps (Copy scale=0.5 -> v = sum_j 4^j*t_j,
    |v| <= 85, exact in int8) + one store DMA per step. Drains are
    emitted several steps late, before the signs, so the in-order ACT
    queue never head-of-line blocks on PE or DVE.
  - Host decodes base-4 balanced digits: t3 = round(v/64), ...,
    out = (t + 1)/2.
  - x-tile pool holds 17 buffers (16 live tiles per chunk + 1 spare) so
    the next chunk's loads start during the current chunk's tail steps.
  Measured on 8 axon trn2 cores (burst differencing, rel err 8.0e-04):
  ~95 us/rep at R=65, 114.1 us at R=17, vs 151.8/144.8 us for the
  previous pack-2 fp8 kernel with per-step ACT drains. At 95 us the
  x-load stream runs at ~355 GB/s/core -- essentially the HBM roofline.
  Dead ends measured on HW: any per-step GPSIMD op (tensor_scalar is
  ~15 ns/col on the Q7, 17x the cost model; and GPSIMD cannot read
  PSUM), and gpsimd-offloaded recurrence slices (1.23 ms/rep).
"""

import numpy as np
import ml_dtypes  # noqa: F401

import concourse.bacc as bacc
import concourse.mybir as mybir
import concourse.tile as tile
import concourse.dve_ops as dve_ops
from concourse.dve_ops import DveOp
from concourse.dve_spec import (
    C0,
    C1,
    C2,
    Spec,
    Src0,
    Src1,
    Zero,
    _has_src1,
    lower,
    select,
)
from concourse.dve_uop import DveOpSpec
from concourse import bass_utils

T, B, N = 16, 64, 65536
NCORES = 8
BS = B // NCORES        # batch rows per core
POS = BS * N            # independent positions per core
P = 128
FREE = POS // P         # 4096 columns per partition
CHUNK = 2048
NCHUNK = FREE // CHUNK
HALF = CHUNK // 2       # 1024
QTR = CHUNK // 4        # 512: packed output columns per chunk
R = 0.75                # fl32(1 / fl32(4/3)) == 0.75 exactly
TH1, TH2 = 0.75, 1.25
SHIFT = TH2 - TH1       # dir-2 state kept as g = h2 - SHIFT
F32 = mybir.dt.float32
BF16 = mybir.dt.bfloat16
FP8 = mybir.dt.float8e4
I8 = mybir.dt.int8
AF = mybir.ActivationFunctionType


def _register(name: str, spec: Spec) -> DveOp:
    """Register a custom DVE op at runtime (uops sha computed here)."""
    if name in dve_ops._SUB_OPCODE_FOR_NAME:
        for op in dve_ops.OPS:
            if op.name == name:
                return op
    row = dve_ops._CUSTOM_DVE_ROW_BASE + len(dve_ops.OPS)
    assert row < 0x20, "custom DVE opcode rows exhausted"
    sha = {}
    for ver in ("v3", "v4"):
        s = DveOpSpec(name=name, opcode=row, uops=lower(spec, ver=ver),
                      rd1_en=_has_src1(spec))
        sha[ver] = s.sha(ver)
    op = DveOp(name, spec, subdim=False, uops_sha=sha)
    dve_ops.OPS.append(op)
    dve_ops._SUB_OPCODE_FOR_NAME[name] = row
    dve_ops.CUSTOM_DVE_SPECS[name] = spec
    return op


# dir 1: h' = (x - vp)*0.75 + vp,  vp = sel(h < th1, h, 0)
_vp1 = select(Src1 < C1, Src1, Zero)
BILIF_STEP = _register(
    "BILIF_STEP",
    Spec(
        body=(Src0 - _vp1) * C0 + _vp1,
        reference=lambda in0, in1, s0, s1, imm2: (
            (in0 - np.where(in1 < s1, in1, 0).astype(np.float32))
            * np.float32(s0)
            + np.where(in1 < s1, in1, 0).astype(np.float32)
        ),
    ),
)

# dir 2, shifted state g = h2 - SHIFT (C2 = -SHIFT):
#   h2_prev = g_prev - C2;  vp = sel(g_prev < C1, g_prev - C2, 0)
#   g' = (x - vp)*C0 + vp + C2
_vp2 = select(Src1 < C1, Src1 - C2, Zero)
BILIF_STEP_S = _register(
    "BILIF_STEP_S",
    Spec(
        body=(Src0 - _vp2) * C0 + _vp2 + C2,
        reference=lambda in0, in1, s0, s1, imm2: (
            (in0 - np.where(in1 < s1, in1 - imm2, 0).astype(np.float32))
            * np.float32(s0)
            + np.where(in1 < s1, in1 - imm2, 0).astype(np.float32)
            + np.float32(imm2)
        ),
    ),
)


def _pack_weights() -> np.ndarray:
    """[128, 512] fp8e4m3, four [128,128] strip tiles W_s. W_s maps
    partition 4q+j -> psum row 32s+q with weight 4**j (all powers of two:
    exact in fp8e4m3, unlike 27), so strip s packs data cols
    [512s, 512s+512) x partition quads into psum rows [32s, 32s+32):
    psum = sum_j 4^j * (sig1+sig2)[4q+j], |psum| <= 170. The drain
    stores v = psum/2 = sum_j 4^j * t_j, |v| <= 85 -- exact in int8."""
    w = np.zeros((128, 512), np.float32)
    for s in range(4):
        for q in range(32):
            for j in range(4):
                w[4 * q + j, 128 * s + 32 * s + q] = 4.0 ** j
    return w.astype(ml_dtypes.float8_e4m3)


_NC_CACHE = {}


def _build_nc(repeat: int = 1):
    """Build + compile the SPMD per-core program. `repeat` replays the body
    (used only for steady-state timing experiments)."""
    key = repeat
    if key in _NC_CACHE:
        return _NC_CACHE[key]
    nc = bacc.Bacc("TRN2", target_bir_lowering=False, debug=False,
                   num_devices=NCORES)
    x_d = nc.dram_tensor("x", [T * P, FREE], F32, kind="ExternalInput").ap()
    w_d = nc.dram_tensor("w", [P, 4 * P], FP8, kind="ExternalInput").ap()
    o_d = nc.dram_tensor("o", [T * P, FREE // 4], I8,
                         kind="ExternalOutput").ap()

    with tile.TileContext(nc) as tc:
        with tc.tile_pool(name="xp", bufs=17) as xp, \
             tc.tile_pool(name="hp", bufs=3) as hp, \
             tc.tile_pool(name="ap", bufs=3) as apool, \
             tc.tile_pool(name="outp", bufs=4) as outp, \
             tc.tile_pool(name="psp", bufs=2, space="PSUM") as psp, \
             tc.tile_pool(name="zp", bufs=1) as zp:
            w4 = []
            for s in range(4):
                ws = zp.tile([P, P], FP8, tag=f"w{s}", name=f"w{s}")
                nc.sync.dma_start(out=ws[:], in_=w_d[:, s * P:(s + 1) * P])
                w4.append(ws)
            b1 = zp.tile([P, 1], F32, tag="b1", name="b1")
            nc.vector.memset(b1[:], -TH1)
            drains = []     # [(ps, t0, c0)] awaiting copy+store; carried
            # across chunk/rep boundaries so the end-of-chunk quads drain
            # AFTER the next chunk's t0 charges (no ACT pileup at the
            # boundary that would stall the next DVE chain).
            for rep in range(repeat):
                for k in range(NCHUNK):
                    c0 = k * CHUNK
                    # Load each x[t] tile once, in first-use order
                    # (fwd uses t at step t, bwd uses t at step 15-t).
                    xt = {}
                    for t in [v for s in range(T // 2) for v in (s, T - 1 - s)]:
                        xt[t] = xp.tile([P, CHUNK], F32, tag="x",
                                        name=f"x{rep}_{k}_{t}")
                        nc.sync.dma_start(
                            out=xt[t][:],
                            in_=x_d[t * P:(t + 1) * P, c0:c0 + CHUNK])
                    h_prev = None
                    ps = None
                    for t in range(T):
                        # h[:, :CHUNK] = h1;  h[:, CHUNK:] = g = h2 - SHIFT
                        h = hp.tile([P, 2 * CHUNK], F32, tag="h", name="h")
                        if t == 0:
                            # v = 0: h1 = .75x, g = .75x' - SHIFT -- on ACT
                            # (keeps the critical DVE chain 2 ops/step)
                            nc.scalar.activation(
                                out=h[:, :CHUNK], in_=xt[0][:],
                                func=AF.Copy, bias=0.0, scale=R)
                            nc.scalar.activation(
                                out=h[:, CHUNK:], in_=xt[T - 1][:],
                                func=AF.Copy, bias=-SHIFT, scale=R)
                        else:
                            nc.vector._custom_dve(
                                BILIF_STEP, out=h[:, :CHUNK], in0=xt[t][:],
                                in1=h_prev[:, :CHUNK], s0=R, s1=TH1)
                            nc.vector._custom_dve(
                                BILIF_STEP_S, out=h[:, CHUNK:],
                                in0=xt[T - 1 - t][:],
                                in1=h_prev[:, CHUNK:], s0=R, s1=TH1,
                                imm2=-SHIFT)
                        # Drain a psum quad late (so the in-order ACT
                        # queue never waits on PE), emitted before the
                        # sign so it never waits on DVE either.
                        if len(drains) > 1:
                            _drain(nc, outp, o_d, drains.pop(0))
                        # One Sign for both dirs: sigma = sign(h - 0.75)
                        a = apool.tile([P, 2 * CHUNK], FP8, tag="a",
                                       name="a")
                        nc.scalar.activation(out=a[:], in_=h[:],
                                             func=AF.Sign, bias=b1[:],
                                             scale=1.0)
                        # Pack-combine 4-to-1: psum[32s+q, f] =
                        # sum_j 4^j * (sig1+sig2)[4q+j, 512s+f] -- 8
                        # accumulating matmuls (4 strips x 2 dirs) per
                        # step into one [128, 512] PSUM bank; four steps
                        # share a 4-bank psum tile so ONE ACT drain op
                        # covers all four.
                        if t % 4 == 0:
                            ps = psp.tile([P, 4 * QTR], F32, tag="ps",
                                          name="ps")
                        ph = slice((t % 4) * QTR, (t % 4) * QTR + QTR)
                        for s in range(4):
                            sa = slice(s * QTR, (s + 1) * QTR)
                            sb = slice(CHUNK + s * QTR, CHUNK + (s + 1) * QTR)
                            nc.tensor.matmul(ps[:, ph], w4[s][:], a[:, sa],
                                             start=(s == 0), stop=False)
                            nc.tensor.matmul(ps[:, ph], w4[s][:], a[:, sb],
                                             start=False, stop=(s == 3))
                        if t % 4 == 3:
                            drains.append((ps, t - 3, c0))
                        h_prev = h
            for d in drains:
                _drain(nc, outp, o_d, d)

    nc.compile()
    _NC_CACHE[key] = nc
    return nc


def _drain(nc, outp, o_d, pending):
    """ACT copy 4-step psum/2 -> int8 (v integer, |v| <= 85: exact),
    then one store per step."""
    ps, t0, c0 = pending
    o = outp.tile([P, 4 * QTR], I8, tag="o", name="o")
    nc.scalar.activation(out=o[:], in_=ps[:], func=AF.Copy,
                         bias=0.0, scale=0.5)
    for i in (0, 1, 2, 3):
        t = t0 + i
        nc.sync.dma_start(
            out=o_d[t * P:(t + 1) * P, c0 // 4:c0 // 4 + QTR],
            in_=o[:, i * QTR:(i + 1) * QTR])


def _run(inputs: np.ndarray, repeat: int = 1, **kwargs):
    nc = _build_nc(repeat)
    w = _pack_weights()
    in_maps = []
    for c in range(NCORES):
        shard = np.ascontiguousarray(
            inputs[:, c * BS:(c + 1) * BS, :]).reshape(T * P, FREE)
        in_maps.append({"x": shard, "w": w})
    return bass_utils.run_bass_kernel_spmd(
        nc, in_maps, core_ids=list(range(NCORES)), **kwargs)


def _decode(o8: np.ndarray) -> np.ndarray:
    """[T*P, FREE//4] int8 packed base-3 -> [T, BS, N] f32 output.

    Packed row 32s+q, col f of chunk k holds v = sum_j 4^j * t_j with
    digits t_j = (sig1+sig2)/2 in {-1,0,1} of partition 4q+j at data col
    512s+f; out = (t+1)/2."""
    w = o8.astype(np.float32).reshape(T, P, NCHUNK, QTR)
    t3 = np.round(w / 64.0)
    r = w - 64.0 * t3
    t2 = np.round(r / 16.0)
    r = r - 16.0 * t2
    t1 = np.round(r / 4.0)
    t0 = r - 4.0 * t1
    digs = (t0, t1, t2, t3)
    out = np.empty((T, P, FREE), np.float32)
    for k in range(NCHUNK):
        for s in range(4):
            rows = slice(32 * s, 32 * s + 32)
            cols = slice(k * CHUNK + s * QTR, k * CHUNK + (s + 1) * QTR)
            for j in range(4):
                out[:, j:P:4, cols] = (digs[j][:, rows, k, :] + 1.0) * 0.5
    return out.reshape(T, BS, N)


def kernel(inputs: np.ndarray, **kwargs) -> np.ndarray:
    inputs = np.asarray(inputs)
    assert inputs.shape == (T, B, N) and inputs.dtype == np.float32
    res = None
    err = None
    for _attempt in range(3):  # retry transient device faults
        try:
            res = _run(inputs, **kwargs)
            break
        except Exception as e:  # noqa: BLE001
            err = e
    if res is None:
        raise err
    out = np.empty((T, B, N), np.float32)
    for c in range(NCORES):
        out[:, c * BS:(c + 1) * BS, :] = _decode(res.results[c]["o"])
    return out

